# revision 19
# baseline (speedup 1.0000x reference)
"""FFD sparse-matmul kernel for Trainium2 (8 NeuronCores).

Problem: out[b, r, d] = sum_i 1[rows_i == r] * vals_i * (x[b, cols_i, d]*scale[d] - offset[d])
  = (A @ xs)[r, j] with xs[k, j] = x[b, k, d]*scale[d] - offset[d], j = b*3+d,
where A is the static [200000, 4096] sparse FFD matrix (12.8M nnz).

Strategy (v2, 4-bit): the dense fp8 weight stream saturated both the 16 SDMA
engines (~330-350 GB/s) and nothing else, so halve the bytes. Weights are
stored as packed 4-bit linear codes (two cells per byte). On-chip, the DVE
expands each byte into two fp8e3m4 weights with a single tensor_scalar per
nibble stream: e3m4's subnormal + first-normal octave is an exactly LINEAR
grid (bit pattern 2n == n/32), so hi = (u16 >> 3) & 0x1E1E and
lo = (u16 << 1) & 0x1E1E decode straight to usable PE weights (measured
~2.9 elem/cyc/partition, ~72us/core for the full matrix). The 32/15 decode
scale folds into xs on the host.

4-bit linear clips cells where duplicate (row,col) nnz sum above 1 (~0.5% of
cells, rel err 8e-2 if ignored). Fix: per 128-row block, the 128
largest-residual cells get an exact fp8 correction via one extra matmul --
stationary [128,128] residual matrix against a host-gathered moving operand
xs[c_e] (residual columns are static). +3% bytes, +3% PE time, and it also
absorbs the worst quantization outliers.

Row-sharded over 8 cores, fp32 PSUM accumulation, GROUP=4 row-blocks per
weight group (~1.1MB). Each group's packed bytes stream as two column-half
dma_starts on BOTH HWDGE queues (one queue serializes consecutive dma_starts
with a ~2us setup gap and tops out ~215 GB/s; partition-sliced weight DMAs
are far worse - they collapse the packet fanout). One PSUM tile per group
(4 block slices), one DVE copy per group emitted two groups late so its
wait-on-PE never delays the expand ops queued behind it. Output stored at
the end in 8 partition-sliced DMAs (SDMA packets are <=64 descriptors and
one packet lands on one engine, so a single 128-row store would serialize
~27us on one engine; 16-partition slices fan out).

Measured trajectory on HW: 379us (fp8 dense baseline) -> 332 (bias fold +
store fix) -> 300 (4-bit GROUP=7) -> 268 (GROUP=4, dual-queue, delayed
copies). PE floor for this shape is ~194us busy (6468 LDW+MM pairs at
~29-30ns); remaining gap is a ~1us/group pipeline handoff artifact.
"""

import os
import numpy as np
import ml_dtypes

N_PTS = 200000
N_CTRL = 4096
B = 2
N_CORES = 8
ROWS_PER_CORE = N_PTS // N_CORES  # 25000
BLK = 128
FN = B * 3  # 6 output columns (batch-major: j = b*3 + d)
GROUP = 4  # row-blocks per weight DMA group
PACK = N_CTRL // 2  # 2048 packed bytes per block row
FIXK = 128  # fixup entries per block
Q = 15.0  # 4-bit code scale: code = round(val*Q) in [0,15], decode n/32

LAST_RESULTS = None

_static_cache = {}
_nc_cache = {}


def _fingerprint(*arrays):
    h = 0
    for a in arrays:
        s = a[:: max(1, a.size // 4096)].tobytes()
        h ^= hash((a.size, s, float(a.astype(np.float64).sum())))
    return h


def _install_profile_shim():
    """Make trace=True work in images whose antenv lacks axon_hooks, and
    neuter the bucket artifact upload. Best-effort; harmless if partial."""
    import sys
    import types

    try:
        import concourse.bass_utils as bu

        bu.upload_artifacts = lambda tmpdir: f"local:{tmpdir}"
    except Exception:
        pass
    try:
        import antenv.axon_hooks  # noqa: F401

        return
    except ImportError:
        pass
    try:
        mod = types.ModuleType("antenv.axon_hooks")
        mod._hook = None
        mod.set_axon_ntff_profile_hook = lambda h: setattr(mod, "_hook", h)
        mod.get_axon_ntff_profile_hook = lambda: mod._hook
        sys.modules["antenv.axon_hooks"] = mod
        import antenv

        antenv.axon_hooks = mod
        if "/root/.axon_site/trn_agent_boot" not in sys.path:
            sys.path.insert(0, "/root/.axon_site/trn_agent_boot")
        from trn_boot import _ntff_profile_via_ctypes

        hook = _ntff_profile_via_ctypes("/opt/axon/libaxon_pjrt.so")
        if hook is not None:
            mod._hook = hook
    except Exception:
        pass


def _build_nc(n_blocks, n_chunks, group=GROUP):
    import concourse.mybir as mybir
    from concourse import bacc
    from concourse.tile import TileContext

    assert n_blocks % group == 0
    f16, f32 = mybir.dt.float16, mybir.dt.float32
    f8, u8, u16 = mybir.dt.float8e3, mybir.dt.uint8, mybir.dt.uint16
    nc = bacc.Bacc()
    n_groups = n_blocks // group
    gpack = group * PACK  # packed bytes per partition per group
    gfix = group * FIXK  # fixup bytes per partition per group
    gw = gpack + gfix  # 15232 for group=7
    xcols = n_chunks * FN + n_blocks * FN  # xs (192) + xfix (1176) fp16

    wG = nc.declare_dram_parameter("wG", [n_groups, BLK, gw], u8, isOutput=False)
    xs = nc.declare_dram_parameter("xs", [BLK, xcols], f16, isOutput=False)
    out = nc.declare_dram_parameter("out", [BLK, n_blocks * FN], f32, isOutput=True)

    shr = mybir.AluOpType.logical_shift_right
    shl = mybir.AluOpType.logical_shift_left
    band = mybir.AluOpType.bitwise_and

    with TileContext(nc) as tc:
        with (
            tc.tile_pool(name="gp", bufs=5) as gp,
            tc.tile_pool(name="ep", bufs=4) as ep,
            tc.tile_pool(name="cp", bufs=1) as cp,
            tc.tile_pool(name="pp", bufs=4, space="PSUM") as pp,
        ):
            # xs + xfix head load, split by partition ranges so its packets
            # spread over engines instead of stalling one engine ~14us.
            x_sb = cp.tile([BLK, xcols], f16, tag="x")
            for i, p0 in enumerate(range(0, BLK, 32)):
                q = nc.sync if i % 2 == 0 else nc.scalar
                q.dma_start(out=x_sb[p0 : p0 + 32, :], in_=xs[p0 : p0 + 32, :])
            xf0 = n_chunks * FN  # xfix column offset inside x_sb

            obuf = cp.tile([BLK, n_blocks * FN], f32, tag="obuf")

            def expand(g_sb, e_sb, c0, c1):
                """DVE-expand packed nibbles [c0:c1) into e_sb hi/lo regions."""
                pin = g_sb[:, c0:c1].bitcast(u16)
                nc.vector.tensor_scalar(
                    out=e_sb[:, c0:c1].bitcast(u16),
                    in0=pin,
                    scalar1=3,
                    scalar2=0x1E1E,
                    op0=shr,
                    op1=band,
                )
                nc.vector.tensor_scalar(
                    out=e_sb[:, gpack + c0 : gpack + c1].bitcast(u16),
                    in0=pin,
                    scalar1=1,
                    scalar2=0x1E1E,
                    op0=shl,
                    op1=band,
                )

            pss = {}

            def group_copy(gj):
                # PSUM -> obuf, one copy per group, emitted two groups late
                # so its wait-on-PE is long satisfied and never delays the
                # DVE queue's expand ops (queue order lockstep otherwise).
                nc.vector.tensor_copy(
                    out=obuf[:, gj * group * FN : (gj + 1) * group * FN],
                    in_=pss.pop(gj)[:],
                )

            for gi in range(n_groups):
                g_sb = gp.tile([BLK, gw], u8, tag="g")
                e_sb = ep.tile([BLK, 2 * gpack], f8, tag="e")
                if gi == 0:
                    # Warmup: per-block chunks so block 0 starts ASAP.
                    for g in range(group):
                        nc.sync.dma_start(
                            out=g_sb[:, g * PACK : (g + 1) * PACK],
                            in_=wG[0][:, g * PACK : (g + 1) * PACK],
                        )
                        expand(g_sb, e_sb, g * PACK, (g + 1) * PACK)
                    nc.sync.dma_start(
                        out=g_sb[:, gpack:], in_=wG[0][:, gpack:]
                    )
                else:
                    # Both HWDGE queues work every group on disjoint COLUMN
                    # halves (full 128-partition descriptor structure is kept
                    # -- partition-sliced weight DMAs serialize badly). One
                    # queue alone tops out ~215 GB/s due to a ~2us setup gap
                    # between consecutive dma_starts.
                    h = gpack // 2
                    qa, qb = (nc.sync, nc.scalar) if gi % 2 == 0 else (nc.scalar, nc.sync)
                    qa.dma_start(out=g_sb[:, :h], in_=wG[gi][:, :h])
                    qb.dma_start(out=g_sb[:, h:], in_=wG[gi][:, h:])
                    # Expand each column half as soon as ITS dma_start lands
                    # (half A fully decodes blocks 0-1) instead of waiting
                    # for the whole group -- shortens the DMA->DVE->PE chain.
                    expand(g_sb, e_sb, 0, h)
                    expand(g_sb, e_sb, h, gpack)
                if gi >= 2:
                    group_copy(gi - 2)
                ps = pp.tile([BLK, group * FN], f32)
                pss[gi] = ps
                for g in range(group):
                    blk = gi * group + g
                    pslice = ps[:, g * FN : (g + 1) * FN]
                    for kc in range(n_chunks):
                        if kc < n_chunks // 2:
                            off = g * PACK + kc * BLK
                        else:
                            off = gpack + g * PACK + (kc - n_chunks // 2) * BLK
                        nc.tensor.matmul(
                            pslice,
                            e_sb[:, off : off + BLK],
                            x_sb[:, kc * FN : (kc + 1) * FN],
                            start=(kc == 0),
                            stop=False,
                        )
                    # Exact fp8 correction for clipped/worst-residual cells.
                    nc.tensor.matmul(
                        pslice,
                        g_sb[:, gpack + g * FIXK : gpack + (g + 1) * FIXK].bitcast(f8),
                        x_sb[:, xf0 + blk * FN : xf0 + (blk + 1) * FN],
                        start=False,
                        stop=True,
                    )
            group_copy(n_groups - 2)
            group_copy(n_groups - 1)
            # Final store, split by partition ranges (8 packets -> 8 engines).
            for i, p0 in enumerate(range(0, BLK, 16)):
                q = nc.sync if i % 2 == 0 else nc.scalar
                q.dma_start(out=out[p0 : p0 + 16, :], in_=obuf[p0 : p0 + 16, :])

    nc.finalize()
    return nc


def _prepare_static(ffd_vals, ffd_rows, ffd_cols):
    """Pack the static sparse matrix into per-core 4-bit groups + fixups."""
    key = _fingerprint(ffd_vals, ffd_rows, ffd_cols)
    if key in _static_cache:
        return _static_cache[key]

    n_blocks = -(-ROWS_PER_CORE // BLK)  # 196
    r_pad = n_blocks * BLK  # 25088
    n_chunks = N_CTRL // BLK  # 32
    n_groups = n_blocks // GROUP

    try:
        from scipy.sparse import coo_matrix

        A = np.asarray(
            coo_matrix(
                (ffd_vals, (ffd_rows, ffd_cols)), shape=(N_PTS, N_CTRL)
            ).todense(),
            dtype=np.float32,
        )
    except Exception:
        A = np.zeros((N_PTS, N_CTRL), np.float32)
        np.add.at(A, (ffd_rows, ffd_cols), ffd_vals)

    wGs, colfixs = [], []
    for c in range(N_CORES):
        Ap = np.zeros((r_pad, N_CTRL), np.float32)
        Ap[:ROWS_PER_CORE] = A[c * ROWS_PER_CORE : (c + 1) * ROWS_PER_CORE]
        codes = np.clip(np.rint(Ap * Q), 0, 15).astype(np.uint8)
        resid = Ap - codes.astype(np.float32) / Q

        # Per block: FIXK worst cells -> exact fp8 fixup matmul. Greedy
        # row-aware selection (give slots to rows with the largest total
        # squared residual) trims the global max error ~7% vs plain top-|R|.
        import heapq

        rb = resid.reshape(n_blocks, BLK, N_CTRL)
        idx = np.empty((n_blocks, FIXK), np.int64)
        TK = 32
        for b in range(n_blocks):
            e = rb[b] * rb[b]
            rowE = e.sum(1)
            ti = np.argpartition(e, -TK, 1)[:, -TK:]
            te = np.take_along_axis(e, ti, 1)
            o = np.argsort(-te, 1)
            ti = np.take_along_axis(ti, o, 1)
            te = np.take_along_axis(te, o, 1)
            ptr = np.zeros(BLK, np.int64)
            heap = [(-rowE[m], m) for m in range(BLK)]
            heapq.heapify(heap)
            for slot in range(FIXK):
                negE, m = heapq.heappop(heap)
                j = ptr[m]
                if j < TK:
                    idx[b, slot] = m * N_CTRL + ti[m, j]
                    heapq.heappush(heap, (negE + te[m, j], m))
                    ptr[m] += 1
                else:
                    idx[b, slot] = m * N_CTRL  # degenerate: resid ~0 cell
                    heapq.heappush(heap, (0.0, m))
        rb = rb.reshape(n_blocks, BLK * N_CTRL)
        rows_f = idx // N_CTRL  # row-in-block of each fixup entry
        cols_f = (idx % N_CTRL).astype(np.int32)  # column of each entry
        vals_f = np.take_along_axis(rb, idx, axis=1)  # residual values
        # F[blk][e, m]: residual at entry e for output row m.
        F = np.zeros((n_blocks, FIXK, BLK), ml_dtypes.float8_e3m4)
        bi = np.repeat(np.arange(n_blocks), FIXK)
        ei = np.tile(np.arange(FIXK), n_blocks)
        F[bi, ei, rows_f.ravel()] = vals_f.ravel().astype(ml_dtypes.float8_e3m4)

        # Stationary layout W[blk][p, kc*BLK + m] = codes[blk*BLK + m, kc*BLK + p]
        w = codes.reshape(n_blocks, BLK, n_chunks, BLK)
        w = w.transpose(0, 3, 2, 1).reshape(n_blocks, BLK, N_CTRL)
        packed = (w[:, :, :PACK] << 4) | w[:, :, PACK:]  # [nb, BLK, 2048] u8

        # Group assembly: [n_groups, BLK, GROUP*PACK + GROUP*FIXK]
        pk = packed.reshape(n_groups, GROUP, BLK, PACK).transpose(0, 2, 1, 3)
        pk = pk.reshape(n_groups, BLK, GROUP * PACK)
        fb = F.view(np.uint8).reshape(n_groups, GROUP, FIXK, BLK)
        fb = fb.transpose(0, 2, 1, 3).reshape(n_groups, FIXK, GROUP * BLK)
        # fixup bytes live on partitions = entry index e (FIXK == BLK)
        wG = np.concatenate([pk, fb], axis=2)
        wGs.append(np.ascontiguousarray(wG))
        colfixs.append(cols_f)  # [n_blocks, FIXK]

    _static_cache.clear()
    _static_cache[key] = (wGs, colfixs)
    return wGs, colfixs


def kernel(x, scale_vec, offset, ffd_vals, ffd_rows, ffd_cols):
    global LAST_RESULTS
    from concourse.bass_utils import run_bass_kernel_spmd

    x = np.asarray(x, np.float32)
    scale_vec = np.asarray(scale_vec, np.float32)
    offset = np.asarray(offset, np.float32)
    ffd_vals = np.asarray(ffd_vals, np.float32)
    ffd_rows = np.asarray(ffd_rows, np.int32)
    ffd_cols = np.asarray(ffd_cols, np.int32)

    n_blocks = -(-ROWS_PER_CORE // BLK)
    r_pad = n_blocks * BLK
    n_chunks = N_CTRL // BLK

    wGs, colfixs = _prepare_static(ffd_vals, ffd_rows, ffd_cols)

    # Dynamic host prep (tiny): xs with offset folded in; device weight grid
    # is n/32 so the main stream's xs carries the 32/Q decode scale.
    x6 = (x * scale_vec[None, None, :] - offset[None, None, :])
    x6 = x6.transpose(1, 0, 2).reshape(N_CTRL, FN).astype(np.float32)
    xmain = (x6 * (32.0 / Q)).astype(np.float16)
    x_tiled = np.ascontiguousarray(
        xmain.reshape(n_chunks, BLK, FN).transpose(1, 0, 2).reshape(BLK, n_chunks * FN)
    )

    in_maps = []
    for c in range(N_CORES):
        # xfix[e, blk*FN + j] = xs_true[cols_f[blk, e], j]  (unscaled)
        xf = x6[colfixs[c]]  # [n_blocks, FIXK, FN] fp32
        xf = xf.transpose(1, 0, 2).reshape(BLK, n_blocks * FN).astype(np.float16)
        xs_full = np.ascontiguousarray(np.concatenate([x_tiled, xf], axis=1))
        in_maps.append({"wG": wGs[c], "xs": xs_full})

    if ("nc", n_blocks, "4bit") not in _nc_cache:
        _nc_cache[("nc", n_blocks, "4bit")] = _build_nc(n_blocks, n_chunks, GROUP)
    nc = _nc_cache[("nc", n_blocks, "4bit")]

    trace = bool(os.environ.get("BASS_TRACE"))
    if trace:
        _install_profile_shim()
    try:
        res = run_bass_kernel_spmd(nc, in_maps, list(range(N_CORES)), trace=trace)
    except Exception:
        if not trace:
            raise
        os.environ.pop("BASS_TRACE", None)
        res = run_bass_kernel_spmd(nc, in_maps, list(range(N_CORES)), trace=False)
    LAST_RESULTS = res

    shards = []
    for c in range(N_CORES):
        o = np.asarray(res.results[c]["out"], np.float32)
        o6 = (
            o.reshape(BLK, n_blocks, FN)
            .transpose(1, 0, 2)
            .reshape(r_pad, FN)[:ROWS_PER_CORE]
        )
        shards.append(o6)
    full6 = np.concatenate(shards, axis=0)
    out = np.ascontiguousarray(
        full6.reshape(N_PTS, B, 3).transpose(1, 0, 2)
    ).astype(np.float32)
    return out


# revision 20
# speedup vs baseline: 1.0621x; 1.0621x over previous
"""FFD sparse-matmul kernel for Trainium2 (8 NeuronCores).

Problem: out[b, r, d] = sum_i 1[rows_i == r] * vals_i * (x[b, cols_i, d]*scale[d] - offset[d])
  = (A @ xs)[r, j] with xs[k, j] = x[b, k, d]*scale[d] - offset[d], j = b*3+d,
where A is the static [200000, 4096] sparse FFD matrix (12.8M nnz).

Strategy (v2, 4-bit): the dense fp8 weight stream saturated both the 16 SDMA
engines (~330-350 GB/s) and nothing else, so halve the bytes. Weights are
stored as packed 4-bit linear codes (two cells per byte). On-chip, the DVE
expands each byte into two fp8e3m4 weights with a single tensor_scalar per
nibble stream: e3m4's subnormal + first-normal octave is an exactly LINEAR
grid (bit pattern 2n == n/32), so hi = (u16 >> 3) & 0x1E1E and
lo = (u16 << 1) & 0x1E1E decode straight to usable PE weights (measured
~2.9 elem/cyc/partition, ~72us/core for the full matrix). The 32/15 decode
scale folds into xs on the host.

4-bit linear clips cells where duplicate (row,col) nnz sum above 1 (~0.5% of
cells, rel err 8e-2 if ignored). Fix: per 128-row block, the 128
largest-residual cells get an exact fp8 correction via one extra matmul --
stationary [128,128] residual matrix against a host-gathered moving operand
xs[c_e] (residual columns are static). +3% bytes, +3% PE time, and it also
absorbs the worst quantization outliers.

Row-sharded over 8 cores, fp32 PSUM accumulation, GROUP=4 row-blocks per
weight group (~1.1MB). Each group's packed bytes stream as two column-half
dma_starts on BOTH HWDGE queues (one queue serializes consecutive dma_starts
with a ~2us setup gap and tops out ~215 GB/s; partition-sliced weight DMAs
are far worse - they collapse the packet fanout). One PSUM tile per group
(4 block slices), one DVE copy per group emitted two groups late so its
wait-on-PE never delays the expand ops queued behind it. Output stored at
the end in 8 partition-sliced DMAs (SDMA packets are <=64 descriptors and
one packet lands on one engine, so a single 128-row store would serialize
~27us on one engine; 16-partition slices fan out).

Measured trajectory on HW: 379us (fp8 dense baseline) -> 332 (bias fold +
store fix) -> 300 (4-bit GROUP=7) -> 268 (GROUP=4, dual-queue, delayed
copies). PE floor for this shape is ~194us busy (6468 LDW+MM pairs at
~29-30ns); remaining gap is a ~1us/group pipeline handoff artifact.
"""

import os
import numpy as np
import ml_dtypes

N_PTS = 200000
N_CTRL = 4096
B = 2
N_CORES = 8
ROWS_PER_CORE = N_PTS // N_CORES  # 25000
BLK = 128
FN = B * 3  # 6 output columns (batch-major: j = b*3 + d)
GROUP = 4  # row-blocks per weight DMA group
PACK = N_CTRL // 2  # 2048 packed bytes per block row
FIXK = 128  # fixup entries per block
Q = 15.0  # 4-bit code scale: code = round(val*Q) in [0,15], decode n/32

LAST_RESULTS = None

_static_cache = {}
_nc_cache = {}


def _fingerprint(*arrays):
    h = 0
    for a in arrays:
        s = a[:: max(1, a.size // 4096)].tobytes()
        h ^= hash((a.size, s, float(a.astype(np.float64).sum())))
    return h


def _install_profile_shim():
    """Make trace=True work in images whose antenv lacks axon_hooks, and
    neuter the bucket artifact upload. Best-effort; harmless if partial."""
    import sys
    import types

    try:
        import concourse.bass_utils as bu

        bu.upload_artifacts = lambda tmpdir: f"local:{tmpdir}"
    except Exception:
        pass
    try:
        import antenv.axon_hooks  # noqa: F401

        return
    except ImportError:
        pass
    try:
        mod = types.ModuleType("antenv.axon_hooks")
        mod._hook = None
        mod.set_axon_ntff_profile_hook = lambda h: setattr(mod, "_hook", h)
        mod.get_axon_ntff_profile_hook = lambda: mod._hook
        sys.modules["antenv.axon_hooks"] = mod
        import antenv

        antenv.axon_hooks = mod
        if "/root/.axon_site/trn_agent_boot" not in sys.path:
            sys.path.insert(0, "/root/.axon_site/trn_agent_boot")
        from trn_boot import _ntff_profile_via_ctypes

        hook = _ntff_profile_via_ctypes("/opt/axon/libaxon_pjrt.so")
        if hook is not None:
            mod._hook = hook
    except Exception:
        pass


def _build_nc(n_blocks, n_chunks, group=GROUP):
    import concourse.mybir as mybir
    from concourse import bacc
    from concourse.tile import TileContext

    assert n_blocks % group == 0
    f16, f32 = mybir.dt.float16, mybir.dt.float32
    f8, u8, u16 = mybir.dt.float8e3, mybir.dt.uint8, mybir.dt.uint16
    nc = bacc.Bacc()
    n_groups = n_blocks // group
    gpack = group * PACK  # packed bytes per partition per group
    gfix = group * FIXK  # fixup bytes per partition per group
    gw = gpack + gfix  # 15232 for group=7
    xcols = n_chunks * FN + n_blocks * FN  # xs (192) + xfix (1176) fp16

    wG = nc.declare_dram_parameter("wG", [n_groups, BLK, gw], u8, isOutput=False)
    xs = nc.declare_dram_parameter("xs", [BLK, xcols], f16, isOutput=False)
    out = nc.declare_dram_parameter("out", [BLK, n_blocks * FN], f32, isOutput=True)

    shr = mybir.AluOpType.logical_shift_right
    shl = mybir.AluOpType.logical_shift_left
    band = mybir.AluOpType.bitwise_and

    with TileContext(nc) as tc:
        with (
            tc.tile_pool(name="gp", bufs=5) as gp,
            tc.tile_pool(name="ep", bufs=4) as ep,
            tc.tile_pool(name="cp", bufs=1) as cp,
            tc.tile_pool(name="pp", bufs=4, space="PSUM") as pp,
        ):
            # xs + xfix head load, split by partition ranges so its packets
            # spread over engines instead of stalling one engine ~14us.
            x_sb = cp.tile([BLK, xcols], f16, tag="x")
            for i, p0 in enumerate(range(0, BLK, 32)):
                q = nc.sync if i % 2 == 0 else nc.scalar
                q.dma_start(out=x_sb[p0 : p0 + 32, :], in_=xs[p0 : p0 + 32, :])
            xf0 = n_chunks * FN  # xfix column offset inside x_sb

            obuf = cp.tile([BLK, n_blocks * FN], f32, tag="obuf")

            def expand(g_sb, e_sb, c0, c1):
                """DVE-expand packed nibbles [c0:c1) into e_sb hi/lo regions."""
                pin = g_sb[:, c0:c1].bitcast(u16)
                nc.vector.tensor_scalar(
                    out=e_sb[:, c0:c1].bitcast(u16),
                    in0=pin,
                    scalar1=3,
                    scalar2=0x1E1E,
                    op0=shr,
                    op1=band,
                )
                nc.vector.tensor_scalar(
                    out=e_sb[:, gpack + c0 : gpack + c1].bitcast(u16),
                    in0=pin,
                    scalar1=1,
                    scalar2=0x1E1E,
                    op0=shl,
                    op1=band,
                )

            pss = {}

            def group_copy(gj):
                # PSUM -> obuf, one copy per group, emitted two groups late
                # so its wait-on-PE is long satisfied and never delays the
                # DVE queue's expand ops (queue order lockstep otherwise).
                nc.vector.tensor_copy(
                    out=obuf[:, gj * group * FN : (gj + 1) * group * FN],
                    in_=pss.pop(gj)[:],
                )

            for gi in range(n_groups):
                g_sb = gp.tile([BLK, gw], u8, tag="g")
                e_sb = ep.tile([BLK, 2 * gpack], f8, tag="e")
                if gi == 0:
                    # Warmup: per-block chunks so block 0 starts ASAP.
                    for g in range(group):
                        nc.sync.dma_start(
                            out=g_sb[:, g * PACK : (g + 1) * PACK],
                            in_=wG[0][:, g * PACK : (g + 1) * PACK],
                        )
                        expand(g_sb, e_sb, g * PACK, (g + 1) * PACK)
                    nc.sync.dma_start(
                        out=g_sb[:, gpack:], in_=wG[0][:, gpack:]
                    )
                else:
                    # Both HWDGE queues work every group on disjoint COLUMN
                    # halves (full 128-partition descriptor structure is kept
                    # -- partition-sliced weight DMAs serialize badly). One
                    # queue alone tops out ~215 GB/s due to a ~2us setup gap
                    # between consecutive dma_starts.
                    h = gpack // 2
                    qa, qb = (nc.sync, nc.scalar) if gi % 2 == 0 else (nc.scalar, nc.sync)
                    qa.dma_start(out=g_sb[:, :h], in_=wG[gi][:, :h])
                    qb.dma_start(out=g_sb[:, h:], in_=wG[gi][:, h:])
                    expand(g_sb, e_sb, 0, gpack)
                if gi >= 2:
                    group_copy(gi - 2)
                ps = pp.tile([BLK, group * FN], f32)
                pss[gi] = ps
                for g in range(group):
                    blk = gi * group + g
                    pslice = ps[:, g * FN : (g + 1) * FN]
                    for kc in range(n_chunks):
                        if kc < n_chunks // 2:
                            off = g * PACK + kc * BLK
                        else:
                            off = gpack + g * PACK + (kc - n_chunks // 2) * BLK
                        nc.tensor.matmul(
                            pslice,
                            e_sb[:, off : off + BLK],
                            x_sb[:, kc * FN : (kc + 1) * FN],
                            start=(kc == 0),
                            stop=False,
                        )
                    # Exact fp8 correction for clipped/worst-residual cells.
                    nc.tensor.matmul(
                        pslice,
                        g_sb[:, gpack + g * FIXK : gpack + (g + 1) * FIXK].bitcast(f8),
                        x_sb[:, xf0 + blk * FN : xf0 + (blk + 1) * FN],
                        start=False,
                        stop=True,
                    )
            group_copy(n_groups - 2)
            group_copy(n_groups - 1)
            # Final store, split by partition ranges (8 packets -> 8 engines).
            for i, p0 in enumerate(range(0, BLK, 16)):
                q = nc.sync if i % 2 == 0 else nc.scalar
                q.dma_start(out=out[p0 : p0 + 16, :], in_=obuf[p0 : p0 + 16, :])

    nc.finalize()
    return nc


def _prepare_static(ffd_vals, ffd_rows, ffd_cols):
    """Pack the static sparse matrix into per-core 4-bit groups + fixups."""
    key = _fingerprint(ffd_vals, ffd_rows, ffd_cols)
    if key in _static_cache:
        return _static_cache[key]

    n_blocks = -(-ROWS_PER_CORE // BLK)  # 196
    r_pad = n_blocks * BLK  # 25088
    n_chunks = N_CTRL // BLK  # 32
    n_groups = n_blocks // GROUP

    try:
        from scipy.sparse import coo_matrix

        A = np.asarray(
            coo_matrix(
                (ffd_vals, (ffd_rows, ffd_cols)), shape=(N_PTS, N_CTRL)
            ).todense(),
            dtype=np.float32,
        )
    except Exception:
        A = np.zeros((N_PTS, N_CTRL), np.float32)
        np.add.at(A, (ffd_rows, ffd_cols), ffd_vals)

    wGs, colfixs = [], []
    for c in range(N_CORES):
        Ap = np.zeros((r_pad, N_CTRL), np.float32)
        Ap[:ROWS_PER_CORE] = A[c * ROWS_PER_CORE : (c + 1) * ROWS_PER_CORE]
        codes = np.clip(np.rint(Ap * Q), 0, 15).astype(np.uint8)
        resid = Ap - codes.astype(np.float32) / Q

        # Per block: FIXK worst cells -> exact fp8 fixup matmul. Greedy
        # row-aware selection (give slots to rows with the largest total
        # squared residual) trims the global max error ~7% vs plain top-|R|.
        import heapq

        rb = resid.reshape(n_blocks, BLK, N_CTRL)
        idx = np.empty((n_blocks, FIXK), np.int64)
        TK = 32
        for b in range(n_blocks):
            e = rb[b] * rb[b]
            rowE = e.sum(1)
            ti = np.argpartition(e, -TK, 1)[:, -TK:]
            te = np.take_along_axis(e, ti, 1)
            o = np.argsort(-te, 1)
            ti = np.take_along_axis(ti, o, 1)
            te = np.take_along_axis(te, o, 1)
            ptr = np.zeros(BLK, np.int64)
            heap = [(-rowE[m], m) for m in range(BLK)]
            heapq.heapify(heap)
            for slot in range(FIXK):
                negE, m = heapq.heappop(heap)
                j = ptr[m]
                if j < TK:
                    idx[b, slot] = m * N_CTRL + ti[m, j]
                    heapq.heappush(heap, (negE + te[m, j], m))
                    ptr[m] += 1
                else:
                    idx[b, slot] = m * N_CTRL  # degenerate: resid ~0 cell
                    heapq.heappush(heap, (0.0, m))
        rb = rb.reshape(n_blocks, BLK * N_CTRL)
        rows_f = idx // N_CTRL  # row-in-block of each fixup entry
        cols_f = (idx % N_CTRL).astype(np.int32)  # column of each entry
        vals_f = np.take_along_axis(rb, idx, axis=1)  # residual values
        # F[blk][e, m]: residual at entry e for output row m.
        F = np.zeros((n_blocks, FIXK, BLK), ml_dtypes.float8_e3m4)
        bi = np.repeat(np.arange(n_blocks), FIXK)
        ei = np.tile(np.arange(FIXK), n_blocks)
        F[bi, ei, rows_f.ravel()] = vals_f.ravel().astype(ml_dtypes.float8_e3m4)

        # Stationary layout W[blk][p, kc*BLK + m] = codes[blk*BLK + m, kc*BLK + p]
        w = codes.reshape(n_blocks, BLK, n_chunks, BLK)
        w = w.transpose(0, 3, 2, 1).reshape(n_blocks, BLK, N_CTRL)
        packed = (w[:, :, :PACK] << 4) | w[:, :, PACK:]  # [nb, BLK, 2048] u8

        # Group assembly: [n_groups, BLK, GROUP*PACK + GROUP*FIXK]
        pk = packed.reshape(n_groups, GROUP, BLK, PACK).transpose(0, 2, 1, 3)
        pk = pk.reshape(n_groups, BLK, GROUP * PACK)
        fb = F.view(np.uint8).reshape(n_groups, GROUP, FIXK, BLK)
        fb = fb.transpose(0, 2, 1, 3).reshape(n_groups, FIXK, GROUP * BLK)
        # fixup bytes live on partitions = entry index e (FIXK == BLK)
        wG = np.concatenate([pk, fb], axis=2)
        wGs.append(np.ascontiguousarray(wG))
        colfixs.append(cols_f)  # [n_blocks, FIXK]

    _static_cache.clear()
    _static_cache[key] = (wGs, colfixs)
    return wGs, colfixs


def kernel(x, scale_vec, offset, ffd_vals, ffd_rows, ffd_cols):
    global LAST_RESULTS
    from concourse.bass_utils import run_bass_kernel_spmd

    x = np.asarray(x, np.float32)
    scale_vec = np.asarray(scale_vec, np.float32)
    offset = np.asarray(offset, np.float32)
    ffd_vals = np.asarray(ffd_vals, np.float32)
    ffd_rows = np.asarray(ffd_rows, np.int32)
    ffd_cols = np.asarray(ffd_cols, np.int32)

    n_blocks = -(-ROWS_PER_CORE // BLK)
    r_pad = n_blocks * BLK
    n_chunks = N_CTRL // BLK

    wGs, colfixs = _prepare_static(ffd_vals, ffd_rows, ffd_cols)

    # Dynamic host prep (tiny): xs with offset folded in; device weight grid
    # is n/32 so the main stream's xs carries the 32/Q decode scale.
    x6 = (x * scale_vec[None, None, :] - offset[None, None, :])
    x6 = x6.transpose(1, 0, 2).reshape(N_CTRL, FN).astype(np.float32)
    xmain = (x6 * (32.0 / Q)).astype(np.float16)
    x_tiled = np.ascontiguousarray(
        xmain.reshape(n_chunks, BLK, FN).transpose(1, 0, 2).reshape(BLK, n_chunks * FN)
    )

    in_maps = []
    for c in range(N_CORES):
        # xfix[e, blk*FN + j] = xs_true[cols_f[blk, e], j]  (unscaled)
        xf = x6[colfixs[c]]  # [n_blocks, FIXK, FN] fp32
        xf = xf.transpose(1, 0, 2).reshape(BLK, n_blocks * FN).astype(np.float16)
        xs_full = np.ascontiguousarray(np.concatenate([x_tiled, xf], axis=1))
        in_maps.append({"wG": wGs[c], "xs": xs_full})

    if ("nc", n_blocks, "4bit") not in _nc_cache:
        _nc_cache[("nc", n_blocks, "4bit")] = _build_nc(n_blocks, n_chunks, GROUP)
    nc = _nc_cache[("nc", n_blocks, "4bit")]

    trace = bool(os.environ.get("BASS_TRACE"))
    if trace:
        _install_profile_shim()
    try:
        res = run_bass_kernel_spmd(nc, in_maps, list(range(N_CORES)), trace=trace)
    except Exception:
        if not trace:
            raise
        os.environ.pop("BASS_TRACE", None)
        res = run_bass_kernel_spmd(nc, in_maps, list(range(N_CORES)), trace=False)
    LAST_RESULTS = res

    shards = []
    for c in range(N_CORES):
        o = np.asarray(res.results[c]["out"], np.float32)
        o6 = (
            o.reshape(BLK, n_blocks, FN)
            .transpose(1, 0, 2)
            .reshape(r_pad, FN)[:ROWS_PER_CORE]
        )
        shards.append(o6)
    full6 = np.concatenate(shards, axis=0)
    out = np.ascontiguousarray(
        full6.reshape(N_PTS, B, 3).transpose(1, 0, 2)
    ).astype(np.float32)
    return out
